# revision 12
# baseline (speedup 1.0000x reference)
"""GAU (Gated Attention Unit) kernel for 8 Trainium2 NeuronCores.

Full inputs in, full output out.  Sharding: data-parallel over batch (4)
x sequence-parallel over output rows (2) = 8 shards, one per core.  Each
core computes k/v for its batch's full sequence and attention outputs for
its half of the rows.  The second-half core receives its tokens rotated by
half the sequence so the device program is identical on every core; the
position-dependent Toeplitz bias is supplied per-core as a precomputed
band table (T[i,j] = g(i-j), a RoPE relative-position identity).
"""

import numpy as np
import ml_dtypes
from contextlib import ExitStack

import concourse.bass as bass
import concourse.bacc as bacc
import concourse.tile as tile
from concourse import mybir
from concourse.bass_utils import run_bass_kernel_spmd
from concourse.masks import make_identity

BF16 = mybir.dt.bfloat16
F32 = mybir.dt.float32
NPBF16 = ml_dtypes.bfloat16

DIM = 512
SH = 128      # shared (qk) dim
EXP = 1024    # expansion dim
PROJ = 2 * EXP + SH  # 2176
LN_EPS = 1e-3
FC = DIM // 128      # feature chunks (4)
PC = PROJ // 128     # proj chunks (17)
NBLK = 512           # n-block width for attention


def _plan(T):
    """Static loop/table geometry for sequence length T."""
    TOWN = T // 2
    MT = T // 128
    NB = TOWN // NBLK
    mhalf = MT // 2
    s0 = lambda mt, nb: nb * NBLK - mt * 128 + T
    sA = [s0(mt, nb) for mt in range(mhalf) for nb in range(NB)]
    sB = [s0(mt, nb) for mt in range(mhalf, MT) for nb in range(NB)]
    baseA, widthA = min(sA), max(sA) + NBLK - min(sA)
    baseB, widthB = min(sB), max(sB) + NBLK - min(sB)
    return dict(T=T, TOWN=TOWN, MT=MT, NB=NB, mhalf=mhalf,
                baseA=baseA, widthA=widthA, baseB=baseB, widthB=widthB)


def _toeplitz_band(a, b, T):
    """g[d], d in [-(T-1), T-1], with T_mat[i, j] = g[i - j + T - 1].

    rope_rows(v, n)[i] = R(theta*i) v pairwise; <R(ti)a, R(tj)b> depends
    only on i-j:  g(d) = sum_f (a1*b1 + a2*b2) cos(d*th_f)
                             + (a1*b2 - a2*b1) sin(d*th_f).
    """
    half = T // 2
    a = np.asarray(a, np.float64)
    b = np.asarray(b, np.float64)
    inv = 10000.0 ** (-(np.arange(half, dtype=np.float64) / half))
    c = a[:half] * b[:half] + a[half:] * b[half:]
    s = a[:half] * b[half:] - a[half:] * b[:half]
    d = np.arange(-(T - 1), T, dtype=np.float64)
    ang = d[:, None] * inv[None, :]
    g = np.cos(ang) @ c + np.sin(ang) @ s
    return g.astype(np.float64)


def _band_tables(g, plan, delta_b):
    """HA/HB tables: H[r, s] = g((s + base) - r - T + delta)."""
    T = plan["T"]
    r = np.arange(128)[:, None]

    def tab(base, width, delta):
        s = np.arange(width)[None, :]
        arg = (s + base) - r - T + delta
        assert arg.min() >= -(T - 1) and arg.max() <= T - 1, (arg.min(), arg.max())
        return g[arg + T - 1].astype(NPBF16)

    ha = tab(plan["baseA"], plan["widthA"], 0)
    hb = tab(plan["baseB"], plan["widthB"], delta_b)
    return ha, hb


def _build_kernel_body(ctx, tc, io, plan, silu_native, spec_beta0,
                       b1v_bc, b2_bc):
    nc = tc.nc
    T, TOWN, MT, NB = plan["T"], plan["TOWN"], plan["MT"], plan["NB"]
    mhalf = plan["mhalf"]
    NTB = T // NBLK       # token blocks of 512 over full seq
    NTBO = TOWN // NBLK   # token blocks over own rows

    SiluF = mybir.ActivationFunctionType.Silu
    SigF = mybir.ActivationFunctionType.Sigmoid
    SqrtF = mybir.ActivationFunctionType.Sqrt
    SquareF = mybir.ActivationFunctionType.Square
    Alu = mybir.AluOpType

    consts = ctx.enter_context(tc.tile_pool(name="consts", bufs=1))
    big32 = ctx.enter_context(tc.tile_pool(name="big32", bufs=1))
    acts = ctx.enter_context(tc.tile_pool(name="acts", bufs=1))
    gpool = ctx.enter_context(tc.tile_pool(name="gpool", bufs=1))
    xstream = ctx.enter_context(tc.tile_pool(name="xstream", bufs=2))
    stats = ctx.enter_context(tc.tile_pool(name="stats", bufs=4))
    sgpool = ctx.enter_context(tc.tile_pool(name="sgpool", bufs=2))
    ostream = ctx.enter_context(tc.tile_pool(name="ostream", bufs=2))
    dram = ctx.enter_context(tc.tile_pool(name="dram", bufs=1, space="DRAM"))
    psmm = ctx.enter_context(
        tc.tile_pool(name="psmm", bufs=2, space=bass.MemorySpace.PSUM))
    psattn = ctx.enter_context(
        tc.tile_pool(name="psattn", bufs=4, space=bass.MemorySpace.PSUM))

    # ---- constants in SBUF ----
    w1_sb = consts.tile([128, FC, PROJ], BF16)
    nc.sync.dma_start(w1_sb, io["w1"].rearrange("(c p) n -> p c n", p=128))
    w2_sb = consts.tile([128, EXP // 128, DIM], BF16)
    nc.sync.dma_start(w2_sb, io["w2"].rearrange("(c p) n -> p c n", p=128))
    b1t_sb = consts.tile([128, PC], F32)
    nc.sync.dma_start(b1t_sb, io["b1t"])
    qkp_sb = consts.tile([128, 4], F32)
    nc.sync.dma_start(qkp_sb, io["qkp"])
    ha_sb = consts.tile([128, plan["widthA"]], BF16)
    nc.sync.dma_start(ha_sb, io["ha"])
    hb_sb = consts.tile([128, plan["widthB"]], BF16)
    nc.sync.dma_start(hb_sb, io["hb"])
    ident = consts.tile([128, 128], BF16)
    make_identity(nc, ident)
    eps_t = consts.tile([128, 1], F32)
    nc.vector.memset(eps_t, LN_EPS)
    if b1v_bc is not None:
        b1v_sb = consts.tile([128, EXP], F32)
        nc.sync.dma_start(b1v_sb, io["b1v"].to_broadcast((128, EXP)))
    if b2_bc is not None:
        b2_sb = consts.tile([128, DIM], F32)
        nc.sync.dma_start(b2_sb, io["b2"].to_broadcast((128, DIM)))

    x_ap = io["x"]
    y_ap = io["y"]

    # ---- phase 0: layernorm (natural layout) -> bf16 scratch ----
    xn_sc = dram.tile([T, DIM], BF16)
    for mt in range(MT):
        xt = xstream.tile([128, DIM], F32, tag="xin")
        nc.sync.dma_start(xt, x_ap[mt * 128:(mt + 1) * 128, :])
        st6 = stats.tile([128, 6], F32)
        nc.vector.bn_stats(st6, xt)
        mv = stats.tile([128, 2], F32)
        nc.vector.bn_aggr(mv, st6)
        rstd = stats.tile([128, 1], F32)
        nc.scalar.activation(rstd, mv[:, 1:2], SqrtF, bias=eps_t, scale=1.0)
        nc.vector.reciprocal(out=rstd, in_=rstd)
        xn = xstream.tile([128, DIM], BF16, tag="xn")
        nc.vector.tensor_scalar(out=xn, in0=xt, scalar1=mv[:, 0:1],
                                scalar2=rstd, op0=Alu.subtract, op1=Alu.mult)
        nc.sync.dma_start(xn_sc[mt * 128:(mt + 1) * 128, :], xn)

    # ---- phase 0b: transposed activations via DMA transpose ----
    xnT = big32.tile([128, FC, T], BF16, tag="big")
    for fc in range(FC):
        nc.sync.dma_start(xnT[:, fc, :], xn_sc[:, fc * 128:(fc + 1) * 128],
                          transpose=True)

    def silu_from_psum(out_ap, ps, bias_col):
        if silu_native:
            if bias_col is None:
                nc.scalar.activation(out_ap, ps, SiluF)
            else:
                nc.scalar.activation(out_ap, ps, SiluF, bias=bias_col, scale=1.0)
        else:
            sg = sgpool.tile([128, out_ap.shape[-1]], BF16, tag="sg")
            if bias_col is None:
                nc.scalar.activation(sg, ps, SigF)
                nc.vector.scalar_tensor_tensor(
                    out=out_ap, in0=ps, scalar=0.0, in1=sg,
                    op0=Alu.add, op1=Alu.mult)
            else:
                nc.scalar.activation(sg, ps, SigF, bias=bias_col, scale=1.0)
                nc.vector.scalar_tensor_tensor(
                    out=out_ap, in0=ps, scalar=bias_col, in1=sg,
                    op0=Alu.add, op1=Alu.mult)

    # ---- phase 1a: v = silu(xn @ W1[:, E:2E])  (natural [token, e]) ----
    v_sb = acts.tile([128, MT, EXP], BF16)
    for mt in range(MT):
        for eb in range(EXP // 512):
            ps = psmm.tile([128, 512], F32)
            for fc in range(FC):
                nc.tensor.matmul(
                    ps,
                    xnT[:, fc, mt * 128:(mt + 1) * 128],
                    w1_sb[:, fc, EXP + eb * 512:EXP + (eb + 1) * 512],
                    start=(fc == 0), stop=(fc == FC - 1))
            if b1v_bc is not None:
                tmp = stats.tile([128, 512], F32, tag="vbias")
                nc.vector.tensor_add(tmp, ps, b1v_sb[:, eb * 512:(eb + 1) * 512])
                silu_from_psum(v_sb[:, mt, eb * 512:(eb + 1) * 512], tmp, None)
            else:
                silu_from_psum(v_sb[:, mt, eb * 512:(eb + 1) * 512], ps, None)

    # ---- phase 1b: uT (transposed, own rows) and baseT (all rows) ----
    uT_sb = acts.tile([128, EXP // 128, TOWN], BF16)
    for pb in range(EXP // 128):
        for tb in range(NTBO):
            ps = psmm.tile([128, 512], F32)
            for fc in range(FC):
                nc.tensor.matmul(
                    ps,
                    w1_sb[:, fc, pb * 128:(pb + 1) * 128],
                    xnT[:, fc, tb * 512:(tb + 1) * 512],
                    start=(fc == 0), stop=(fc == FC - 1))
            silu_from_psum(uT_sb[:, pb, tb * 512:(tb + 1) * 512], ps,
                           b1t_sb[:, pb:pb + 1])

    baseT = acts.tile([128, T], BF16)
    for tb in range(NTB):
        ps = psmm.tile([128, 512], F32)
        for fc in range(FC):
            nc.tensor.matmul(
                ps,
                w1_sb[:, fc, 2 * EXP:2 * EXP + 128],
                xnT[:, fc, tb * 512:(tb + 1) * 512],
                start=(fc == 0), stop=(fc == FC - 1))
        silu_from_psum(baseT[:, tb * 512:(tb + 1) * 512], ps,
                       b1t_sb[:, 2 * EXP // 128:2 * EXP // 128 + 1])

    # ---- phase 1c: q/k offset-scale ----
    # qT = baseT[:, :TOWN] * qkp[:,0] + qkp[:,1]; spec_beta0 folds both
    # gammas (and the 1/T qk scale) into the q side so kT = baseT as-is.
    qT = acts.tile([128, TOWN], BF16)
    nc.vector.tensor_scalar(out=qT, in0=baseT[:, :TOWN],
                            scalar1=qkp_sb[:, 0:1], scalar2=qkp_sb[:, 1:2],
                            op0=Alu.mult, op1=Alu.add)
    if not spec_beta0:
        nc.vector.tensor_scalar(out=baseT, in0=baseT,
                                scalar1=qkp_sb[:, 2:3], scalar2=qkp_sb[:, 3:4],
                                op0=Alu.mult, op1=Alu.add)
    kT = baseT

    # ---- phase 2/3: attention + gate + proj2, per n-block ----
    for nb in range(NB):
        sT = big32.tile([128, MT, NBLK], BF16, tag="big")
        for mt in range(MT):
            ps = psmm.tile([128, NBLK], F32)
            s0 = nb * NBLK - mt * 128 + T
            if mt < mhalf:
                hsl = ha_sb[:, s0 - plan["baseA"]:s0 - plan["baseA"] + NBLK]
            else:
                hsl = hb_sb[:, s0 - plan["baseB"]:s0 - plan["baseB"] + NBLK]
            nc.tensor.matmul(ps, ident, hsl, start=True, stop=False)
            nc.tensor.matmul(ps, kT[:, mt * 128:(mt + 1) * 128],
                             qT[:, nb * NBLK:(nb + 1) * NBLK],
                             start=False, stop=True)
            # relu(x)^2: DVE max(x,0) PSUM->SBUF, then ACT square
            # (a single STT reading ps twice is rejected by neuronx-cc:
            # only one non-scalar input may come from PSUM)
            zr = sgpool.tile([128, NBLK], BF16, tag="sg")
            nc.vector.tensor_scalar_max(out=zr, in0=ps, scalar1=0.0)
            nc.scalar.activation(sT[:, mt, :], zr, SquareF)

        gT = gpool.tile([128, EXP // 128, NBLK], BF16, tag="gT")
        for wave in range(2):
            pas = []
            for e4 in range(4):
                pa = psattn.tile([128, NBLK], F32, tag="pa")
                pas.append(pa)
            for mt in range(MT):
                for e4 in range(4):
                    ec = wave * 4 + e4
                    nc.tensor.matmul(
                        pas[e4],
                        v_sb[:, mt, ec * 128:(ec + 1) * 128],
                        sT[:, mt, :],
                        start=(mt == 0), stop=(mt == MT - 1))
            for e4 in range(4):
                ec = wave * 4 + e4
                nc.vector.tensor_mul(
                    gT[:, ec, :], pas[e4],
                    uT_sb[:, ec, nb * NBLK:(nb + 1) * NBLK])

        for nt in range(NBLK // 128):
            rows = nb * NBLK + nt * 128
            psy = psmm.tile([128, DIM], F32)
            for ec in range(EXP // 128):
                nc.tensor.matmul(
                    psy, gT[:, ec, nt * 128:(nt + 1) * 128],
                    w2_sb[:, ec, :],
                    start=(ec == 0), stop=(ec == EXP // 128 - 1))
            xs = ostream.tile([128, DIM], F32, tag="xs")
            nc.sync.dma_start(xs, x_ap[rows:rows + 128, :])
            ys = ostream.tile([128, DIM], F32, tag="ys")
            if b2_bc is not None:
                nc.vector.tensor_add(ys, psy, b2_sb)
                nc.vector.tensor_add(ys, ys, xs)
            else:
                nc.vector.tensor_add(ys, psy, xs)
            nc.sync.dma_start(y_ap[rows:rows + 128, :], ys)


_PROG_CACHE = {}


def _get_program(T, silu_native, spec_beta0, with_b1v, with_b2, repeats=1):
    key = (T, silu_native, spec_beta0, with_b1v, with_b2, repeats)
    if key in _PROG_CACHE:
        return _PROG_CACHE[key]
    plan = _plan(T)
    nc = bacc.Bacc("TRN2", target_bir_lowering=False, debug=False)
    io = {
        "x": nc.dram_tensor("x", [T, DIM], F32, kind="ExternalInput").ap(),
        "w1": nc.dram_tensor("w1", [DIM, PROJ], BF16, kind="ExternalInput").ap(),
        "w2": nc.dram_tensor("w2", [EXP, DIM], BF16, kind="ExternalInput").ap(),
        "b1t": nc.dram_tensor("b1t", [128, PC], F32, kind="ExternalInput").ap(),
        "qkp": nc.dram_tensor("qkp", [128, 4], F32, kind="ExternalInput").ap(),
        "ha": nc.dram_tensor("ha", [128, plan["widthA"]], BF16,
                             kind="ExternalInput").ap(),
        "hb": nc.dram_tensor("hb", [128, plan["widthB"]], BF16,
                             kind="ExternalInput").ap(),
        "y": nc.dram_tensor("y", [plan["TOWN"], DIM], F32,
                            kind="ExternalOutput").ap(),
    }
    if with_b1v:
        io["b1v"] = nc.dram_tensor("b1v", [1, EXP], F32,
                                   kind="ExternalInput").ap()
    if with_b2:
        io["b2"] = nc.dram_tensor("b2", [1, DIM], F32,
                                  kind="ExternalInput").ap()
    with tile.TileContext(nc) as tc:
        for _ in range(repeats):
            with ExitStack() as ctx:
                _build_kernel_body(ctx, tc, io, plan, silu_native, spec_beta0,
                                   "b1v" if with_b1v else None,
                                   "b2" if with_b2 else None)
    nc.compile()
    _PROG_CACHE[key] = (nc, plan)
    return nc, plan


def prepare_in_maps(x, ln_gamma, ln_beta, W1, b1, W2, b2, a, b, gamma, beta,
                    silu_native=True, repeats=1):
    """Host-side prep: fold LN affine + qk scale into weights, build the
    Toeplitz band tables, shard per core.  Returns (nc, plan, in_maps, B)."""
    x = np.asarray(x, np.float32)
    B, T, _ = x.shape
    W1 = np.asarray(W1, np.float64)
    W1eff = np.asarray(ln_gamma, np.float64)[:, None] * W1
    b1eff = np.asarray(ln_beta, np.float64) @ W1 + np.asarray(b1, np.float64)
    w1_bf = W1eff.astype(np.float32).astype(NPBF16)
    w2_bf = np.asarray(W2, np.float32).astype(NPBF16)
    b1t = np.ascontiguousarray(
        b1eff.astype(np.float32).reshape(PC, 128).T)

    gamma = np.asarray(gamma, np.float64)
    beta = np.asarray(beta, np.float64)
    spec_beta0 = bool(np.all(beta == 0.0))
    qkp = np.zeros((128, 4), np.float32)
    if spec_beta0:
        qkp[:, 0] = (gamma[0] * gamma[1] / T).astype(np.float32)
    else:
        qkp[:, 0] = (gamma[0] / T).astype(np.float32)
        qkp[:, 1] = (beta[0] / T).astype(np.float32)
        qkp[:, 2] = gamma[1].astype(np.float32)
        qkp[:, 3] = beta[1].astype(np.float32)

    b1v = np.asarray(b1, np.float32)[EXP:2 * EXP]
    with_b1v = bool(np.any(b1v != 0.0))
    b2 = np.asarray(b2, np.float32)
    with_b2 = bool(np.any(b2 != 0.0))

    nc, plan = _get_program(T, silu_native, spec_beta0, with_b1v, with_b2)

    g = _toeplitz_band(a, b, T)
    ha0, hb0 = _band_tables(g, plan, 0)      # first-half cores
    _, hb1 = _band_tables(g, plan, T)        # second-half cores

    in_maps = []
    for core in range(2 * B):
        bidx, h = core // 2, core % 2
        if h == 0:
            xc = x[bidx]
        else:
            xc = np.concatenate([x[bidx, T // 2:], x[bidx, :T // 2]], axis=0)
        m = {"x": np.ascontiguousarray(xc), "w1": w1_bf, "w2": w2_bf,
             "b1t": b1t, "qkp": qkp, "ha": ha0, "hb": hb0 if h == 0 else hb1}
        if with_b1v:
            m["b1v"] = b1v.reshape(1, EXP)
        if with_b2:
            m["b2"] = b2.reshape(1, DIM)
        in_maps.append(m)
    return nc, plan, in_maps, B


def kernel(x, ln_gamma, ln_beta, W1, b1, W2, b2, a, b, gamma, beta):
    x = np.asarray(x, np.float32)
    B, T, D = x.shape
    nc, plan, in_maps, _ = prepare_in_maps(
        x, ln_gamma, ln_beta, W1, b1, W2, b2, a, b, gamma, beta)
    res = run_bass_kernel_spmd(nc, in_maps, list(range(2 * B)))
    out = np.empty((B, T, D), np.float32)
    TOWN = T // 2
    for core in range(2 * B):
        bidx, h = core // 2, core % 2
        out[bidx, h * TOWN:(h + 1) * TOWN] = res.results[core]["y"]
    return out
